# revision 17
# baseline (speedup 1.0000x reference)
"""Trainium2 Bass kernel for nn_BASE_MAMBA_14018773254552.

Time-half pipelined variant: front-end and scan phase split into t-halves;
FE(h1) hides under the half-0 scan phase; group-merged half-1 scans get
per-pair carries via break-column injection (dA=0, dBx=carry).

Front-end: the depthwise causal conv is folded into the W_in projection as
4 PSUM-accumulating matmuls with host-premultiplied weights
diag(conv_w[:,k]) @ W_in over shifted h slices; FE datapath runs in bf16.
PSUM->SBUF casts ride on the Act engine (Copy is table-load-free).
"""
import numpy as np

try:
    import concourse.bacc as bacc
except ImportError:  # pragma: no cover - path fallback
    import sys
    for _p in ("/opt/trn_rl_repo", "/root/.axon_site/_ro/trn_rl_repo"):
        if _p not in sys.path:
            sys.path.insert(0, _p)
    import concourse.bacc as bacc

import ml_dtypes
import concourse.bass as bass
import concourse.mybir as mybir
import concourse.tile as tile
from concourse.bass_utils import run_bass_kernel_spmd

F32 = mybir.dt.float32
BF16 = mybir.dt.bfloat16
AF = mybir.ActivationFunctionType
OP = mybir.AluOpType

B, L, CIN = 4, 1024, 20
DM, DS, DC = 128, 64, 4
DI = 256
DTR = 8
DH = 128
NP = DH // 2
G = 4
HF = 512          # t-half length
LS = HF + 2
GW = G * LS - 2
NG = NP // G
EPS = 1e-5

_cache = {}


def _build():
    nc = bacc.Bacc("TRN2", target_bir_lowering=False, debug=False, num_devices=8)

    xt_d = nc.dram_tensor("xt", [CIN, L], F32, kind="ExternalInput")
    wpT_d = nc.dram_tensor("wpT", [CIN, DM], F32, kind="ExternalInput")
    bp_d = nc.dram_tensor("bp", [DM, 1], F32, kind="ExternalInput")
    # 9 folded [DM, DH] blocks: 4 conv taps x own, 4 x other, z
    wiT_d = nc.dram_tensor("wiT", [DM, 9 * DH], BF16, kind="ExternalInput")
    convb_d = nc.dram_tensor("convb", [DH, 2], F32, kind="ExternalInput")
    wxT_d = nc.dram_tensor("wxT", [DH, 2 * 136], BF16, kind="ExternalInput")
    wdtT_d = nc.dram_tensor("wdtT", [DTR, DH], BF16, kind="ExternalInput")
    bdt_d = nc.dram_tensor("bdt", [DH, 1], F32, kind="ExternalInput")
    alogp_d = nc.dram_tensor("alogp", [DH, DS], F32, kind="ExternalInput")
    dskip_d = nc.dram_tensor("dskip", [DH, 1], F32, kind="ExternalInput")
    woutT_d = nc.dram_tensor("woutT", [DH, DM], BF16, kind="ExternalInput")
    selE_d = nc.dram_tensor("selE", [DH, DS * DH], BF16, kind="ExternalInput")
    selR_d = nc.dram_tensor("selR", [DH, DS * DH], BF16, kind="ExternalInput")
    pooled_d = nc.dram_tensor("pooled", [DM, 1], F32, kind="ExternalOutput")
    u_scr = nc.dram_tensor("u_scr", [DH, L], BF16)
    bm_scr = nc.dram_tensor("bm_scr", [DS, L], BF16)
    cm_scr = nc.dram_tensor("cm_scr", [DS, L], BF16)

    HLF = (slice(0, HF), slice(HF, L))

    with tile.TileContext(nc) as tc:
        with (
            tc.tile_pool(name="const", bufs=1) as cp,
            tc.tile_pool(name="work", bufs=1) as wp,
            tc.tile_pool(name="fe", bufs=1) as fe,
        ):
            xt = cp.tile([CIN, L], F32)
            wpT = cp.tile([CIN, DM], F32)
            bp = cp.tile([DM, 1], F32)
            wiT = cp.tile([DM, 9 * DH], BF16)
            convb = cp.tile([DH, 2], F32)
            wxT = cp.tile([DH, 2 * 136], BF16)
            wdtT = cp.tile([DTR, DH], BF16)
            bdt = cp.tile([DH, 1], F32)
            alogp = cp.tile([DH, DS], F32)
            dskip = cp.tile([DH, 1], F32)
            woutT = cp.tile([DH, DM], BF16)
            selE = cp.tile([DH, DS * DH], BF16)
            selR = cp.tile([DH, DS * DH], BF16)
            engs = [nc.sync, nc.scalar]
            for n_, (t_, d_) in enumerate(
                          [(xt, xt_d), (wpT, wpT_d), (bp, bp_d), (wiT, wiT_d),
                           (convb, convb_d), (wxT, wxT_d),
                           (wdtT, wdtT_d), (bdt, bdt_d), (alogp, alogp_d),
                           (dskip, dskip_d), (woutT, woutT_d)]):
                engs[n_ % 2].dma_start(t_[:], d_[:])
            for t_, d_ in ((selR, selR_d), (selE, selE_d)):
                for c in range(8):
                    csl = slice(c * DS * DH // 8, (c + 1) * DS * DH // 8)
                    nc.sync.dma_start(t_[:, csl], d_[:, csl])

            xc0 = wp.tile([DH, L], BF16, tag="xc0")
            xc1 = wp.tile([DH, L], BF16, tag="xc1")
            xc16 = [xc0, xc1]
            zsig = wp.tile([DH, L], BF16)
            DT = wp.tile([DH, L], BF16)
            U = wp.tile([DH, L], BF16)
            bmT16 = wp.tile([DS, L], BF16)
            cmT16 = wp.tile([DS, L], BF16)
            aposp = wp.tile([DH, DS], F32)
            carryT = wp.tile([DH, NP], BF16)
            Bm2h0 = wp.tile([DH, GW], BF16, tag="Bm2h0")
            Bm2h1 = wp.tile([DH, GW], BF16, tag="Bm2h1")
            Cm2h0 = wp.tile([DH, GW], BF16, tag="Cm2h0")
            Cm2h1 = wp.tile([DH, GW], BF16, tag="Cm2h1")
            BmH = [Bm2h0, Bm2h1]
            CmH = [Cm2h0, Cm2h1]

            # h padded with DC-1 leading zero columns for the folded conv
            h16 = fe.tile([DM, DC - 1 + L], BF16)
            nc.vector.memset(h16[:, 0:DC - 1], 0.0)
            dtrT = fe.tile([DTR, L], BF16)
            sg = fe.tile([DH, L], F32)
            bdtn = fe.tile([DH, 1], F32)
            nc.scalar.mul(bdtn[:], bdt[:], -1.0)
            nc.scalar.activation(aposp[:], alogp[:], AF.Exp)

            y2 = wp.tile([DH, L], BF16)
            y3 = wp.tile([DH, L], BF16)
            pooled_h = wp.tile([DM, 2], F32)
            pooled = wp.tile([DM, 1], F32)

            with (
                tc.tile_pool(name="psl", bufs=1, space="PSUM") as psl,
                tc.tile_pool(name="psr", bufs=2, space="PSUM") as psr,
                tc.tile_pool(name="pre", bufs=3) as prep,
                tc.tile_pool(name="da", bufs=2) as dap,
                tc.tile_pool(name="sl", bufs=5) as slp,
                tc.tile_pool(name="ps2", bufs=1, space="PSUM") as ps2,
            ):
                Y_ps = psl.tile([DH, L], F32, tag="Y")
                u_sap = u_scr[:]

                def fe_half(hh, ps1):
                    sl = HLF[hh]
                    h_ps = ps1.tile([DM, HF], F32, tag="ps")
                    nc.tensor.matmul(h_ps[:], wpT[:, :], xt[:, sl])
                    nc.scalar.activation(h16[:, sl.start + 3:sl.stop + 3],
                                         h_ps[:], AF.Identity, bias=bp[:])
                    for j in range(2):
                        xc_ps = ps1.tile([DH, HF], F32, tag="ps")
                        for k in range(DC):
                            nc.tensor.matmul(
                                xc_ps[:],
                                wiT[:, (4 * j + k) * DH:(4 * j + k + 1) * DH],
                                h16[:, sl.start + k:sl.start + k + HF],
                                start=(k == 0), stop=(k == DC - 1))
                        nc.scalar.activation(xc16[j][:, sl], xc_ps[:], AF.Silu,
                                             bias=convb[:, j:j + 1])
                    # z-gate here so all silus share one act-table residency;
                    # only the tail consumes zsig.
                    z_ps = ps1.tile([DH, HF], F32, tag="ps")
                    nc.tensor.matmul(z_ps[:], wiT[:, 8 * DH:9 * DH],
                                     h16[:, sl.start + 3:sl.stop + 3])
                    nc.scalar.activation(zsig[:, sl], z_ps[:], AF.Silu)
                    dtr_ps = ps1.tile([DTR, HF], F32, tag="ps")
                    bc_ps = ps1.tile([2 * DS, HF], F32, tag="ps")
                    for (m0, msz, out_ps) in ((0, DTR, dtr_ps),
                                              (DTR, 2 * DS, bc_ps)):
                        for j in range(2):
                            nc.tensor.matmul(
                                out_ps[:],
                                wxT[:, 136 * j + m0:136 * j + m0 + msz],
                                xc16[j][:, sl],
                                start=(j == 0), stop=(j == 1))
                    nc.scalar.activation(dtrT[:, sl], dtr_ps[:], AF.Copy)
                    nc.scalar.activation(bmT16[:, sl], bc_ps[0:DS, :], AF.Copy)
                    nc.scalar.activation(cmT16[:, sl], bc_ps[DS:2 * DS, :],
                                         AF.Copy)

                    dt_ps = ps1.tile([DH, HF], F32, tag="ps")
                    nc.tensor.matmul(dt_ps[:], wdtT[:, :], dtrT[:, sl])
                    # DT = -dt = ln(sigmoid(-(raw+bdt))); sign folded into U
                    nc.scalar.activation(sg[:, sl], dt_ps[:], AF.Sigmoid,
                                         bias=bdtn[:], scale=-1.0)
                    nc.scalar.activation(DT[:, sl], sg[:, sl], AF.Ln)
                    nc.vector.scalar_tensor_tensor(
                        out=U[:, sl], in0=DT[:, sl], scalar=-1.0,
                        in1=xc16[0][:, sl], op0=OP.mult, op1=OP.mult)
                    nc.sync.dma_start(u_scr[:, sl], U[:, sl])
                    nc.sync.dma_start(bm_scr[:, sl], bmT16[:, sl])
                    nc.sync.dma_start(cm_scr[:, sl], cmT16[:, sl])
                    for scr, dst in ((bm_scr, BmH[hh]), (cm_scr, CmH[hh])):
                        sap = scr[:]
                        nc.vector.memset(bass.AP(
                            tensor=dst[:].tensor, offset=dst[:].offset + HF,
                            ap=[dst[:].ap[0], [LS, G - 1], [1, 2]]), 0.0)
                        for k in range(G):
                            nc.sync.dma_start(
                                dst[:, k * LS:k * LS + HF],
                                bass.AP(tensor=sap.tensor,
                                        offset=sap.offset + sl.start,
                                        ap=[sap.ap[0], [0, 2], [1, HF]]))
                dA_q = {}

                def p2_pre(hh, g):
                    # prefetch stage: urep DMA + dtrep matmuls + exps; emitted
                    # one group ahead so PE's dtreps precede group g-1's Y
                    # matmuls in queue order.
                    sl = HLF[hh]
                    urep = prep.tile([DH, GW], BF16, tag=f"urep{hh}")
                    if g < 3:
                        nc.vector.memset(bass.AP(
                            tensor=urep[:].tensor, offset=urep[:].offset + HF,
                            ap=[urep[:].ap[0], [LS, G - 1], [1, 2]]), 0.0)
                    for k in range(G):
                        p = g * G + k
                        nc.sync.dma_start(
                            urep[:, k * LS:k * LS + HF],
                            bass.AP(tensor=u_sap.tensor,
                                    offset=u_sap.offset + p * 2 * L + sl.start,
                                    ap=[[0, DS], [L, 2], [1, HF]]))
                    dA = dap.tile([DH, GW], BF16, tag=f"dA{hh}")
                    if g < 2:
                        if hh == 0:
                            nc.vector.memset(bass.AP(
                                tensor=dA[:].tensor, offset=dA[:].offset + HF,
                                ap=[dA[:].ap[0], [LS, G - 1], [1, 2]]), 0.0)
                        else:
                            nc.vector.memset(bass.AP(
                                tensor=dA[:].tensor, offset=dA[:].offset + HF,
                                ap=[dA[:].ap[0], [LS, G - 1], [1, 1]]), 0.0)
                            nc.vector.memset(bass.AP(
                                tensor=dA[:].tensor,
                                offset=dA[:].offset + HF + 1,
                                ap=[dA[:].ap[0], [LS, G - 1], [1, 1]]), 1.0)
                    for k in range(G):
                        p = g * G + k
                        dtrep = psr.tile([DH, HF], F32, tag="dtrep")
                        nc.tensor.matmul(dtrep[:],
                                         selR[:, DH * p:DH * (p + 1)],
                                         DT[:, sl])
                        nc.scalar.activation(
                            dA[:, k * LS:k * LS + HF], dtrep[:], AF.Exp,
                            scale=aposp[:, p:p + 1])
                    dA_q[(hh, g)] = (urep, dA)

                def p2_main(hh, g):
                    sl = HLF[hh]
                    urep, dA = dA_q.pop((hh, g))
                    dBx = slp.tile([DH, GW], BF16, tag="dBx")
                    if hh == 1:
                        # inject pair k's half-0 final state at its leading
                        # break col BEFORE the strided TT (no WAR on drain);
                        # the TT below skips break cols.
                        dsap = dBx[:]
                        nc.vector.tensor_copy(
                            bass.AP(tensor=dsap.tensor,
                                    offset=dsap.offset + HF,
                                    ap=[dsap.ap[0], [LS, G - 1], [1, 1]]),
                            carryT[:, g * G + 1:g * G + G])
                        blk = lambda t: bass.AP(
                            tensor=t[:].tensor, offset=t[:].offset,
                            ap=[t[:].ap[0], [LS, G], [1, HF]])
                        nc.vector.tensor_tensor(out=blk(dBx), in0=blk(urep),
                                                in1=blk(BmH[hh]), op=OP.mult)
                    else:
                        # full-width: break cols become urep_break*0 = 0
                        nc.vector.tensor_tensor(out=dBx[:], in0=urep[:],
                                                in1=BmH[hh][:], op=OP.mult)
                    Ht = slp.tile([DH, GW], BF16, tag="H")
                    init = 0.0 if hh == 0 else carryT[:, g * G:g * G + 1]
                    nc.vector.tensor_tensor_scan(
                        out=Ht[:], data0=dA[:], data1=dBx[:], initial=init,
                        op0=OP.mult, op1=OP.add)
                    HCt = slp.tile([DH, GW], BF16, tag="HC")
                    # front 3/4 on Pool (off DVE's critical chain), tail on
                    # DVE so the last Y matmul isn't gated on Pool latency
                    nc.gpsimd.tensor_tensor(out=HCt[:, 0:3 * LS],
                                            in0=Ht[:, 0:3 * LS],
                                            in1=CmH[hh][:, 0:3 * LS],
                                            op=OP.mult)
                    nc.vector.tensor_tensor(out=HCt[:, 3 * LS:GW],
                                            in0=Ht[:, 3 * LS:GW],
                                            in1=CmH[hh][:, 3 * LS:GW],
                                            op=OP.mult)
                    if hh == 0:
                        hsap = Ht[:]
                        nc.vector.tensor_copy(
                            carryT[:, g * G:g * G + G],
                            bass.AP(tensor=hsap.tensor,
                                    offset=hsap.offset + HF - 1,
                                    ap=[hsap.ap[0], [LS, G], [1, 1]]))
                    for k in range(G):
                        p = g * G + k
                        nc.tensor.matmul(
                            Y_ps[:, sl], selE[:, DH * p:DH * (p + 1)],
                            HCt[:, k * LS:k * LS + HF],
                            start=(p == 0), stop=(p == NP - 1))

                def tail_half(hh):
                    sl = HLF[hh]
                    nc.vector.scalar_tensor_tensor(
                        out=y2[:, sl], in0=xc16[0][:, sl], scalar=dskip[:],
                        in1=Y_ps[:, sl], op0=OP.mult, op1=OP.add)
                    nc.vector.tensor_tensor(out=y3[:, sl], in0=y2[:, sl],
                                            in1=zsig[:, sl], op=OP.mult)
                    out_ps = ps2.tile([DM, HF], F32, tag="o")
                    nc.tensor.matmul(out_ps[:], woutT[:, :], y3[:, sl])
                    nc.vector.tensor_reduce(
                        out=pooled_h[:, hh:hh + 1], in_=out_ps[:],
                        op=OP.add, axis=mybir.AxisListType.X)

                with tc.tile_pool(name="ps1", bufs=3, space="PSUM") as ps1:
                    fe_half(0, ps1)
                    p2_pre(0, 0)
                    p2_pre(0, 1)
                    p2_main(0, 0)
                    fe_half(1, ps1)
                for g in range(2, NG):
                    p2_pre(0, g)
                    p2_main(0, g - 1)
                p2_pre(1, 0)
                p2_main(0, NG - 1)
                tail_half(0)
                for g in range(1, NG):
                    p2_pre(1, g)
                    p2_main(1, g - 1)
                p2_main(1, NG - 1)
                tail_half(1)
                nc.vector.tensor_tensor(
                    out=pooled[:], in0=pooled_h[:, 0:1],
                    in1=pooled_h[:, 1:2], op=OP.add)
                nc.sync.dma_start(pooled_d[:], pooled[:])

    nc.compile()
    return nc


def _core_inputs(inputs, b, half):
    f32 = np.float32
    bf16 = ml_dtypes.bfloat16
    x = np.asarray(inputs["x"], f32)
    Wp = np.asarray(inputs["Wp"], f32)
    bp = np.asarray(inputs["bp"], f32)
    W_in = np.asarray(inputs["W_in"], f32)
    conv_w = np.asarray(inputs["conv_w"], f32)
    conv_b = np.asarray(inputs["conv_b"], f32)
    W_x = np.asarray(inputs["W_x"], f32)
    W_dt = np.asarray(inputs["W_dt"], f32)
    b_dt = np.asarray(inputs["b_dt"], f32)
    A_log = np.asarray(inputs["A_log"], f32)
    Dskip = np.asarray(inputs["Dskip"], f32)
    W_out = np.asarray(inputs["W_out"], f32)

    own = slice(half * DH, half * DH + DH)
    other = slice(DH, 2 * DH) if half == 0 else slice(0, DH)
    # folded conv taps: (diag(conv_w[:,k]) @ W_in_part).T = W_in_part.T * w_k
    wiT_blocks = []
    for part in (own, other):
        Wpart = W_in[0:DI][part]                   # [DH, DM]
        for k in range(DC):
            wiT_blocks.append((Wpart * conv_w[part][:, k:k + 1]).T)
    wiT_blocks.append(W_in[DI:2 * DI][own].T)
    return {
        "xt": np.ascontiguousarray(x[b]),
        "wpT": np.ascontiguousarray(Wp.T),
        "bp": np.ascontiguousarray(bp[:, None]),
        "wiT": np.concatenate(wiT_blocks, axis=1).astype(bf16),
        "convb": np.stack([conv_b[own], conv_b[other]], axis=1),
        "wxT": np.concatenate([W_x.T[own], W_x.T[other]],
                              axis=1).astype(bf16),
        "wdtT": np.ascontiguousarray(W_dt[own].T).astype(bf16),
        "bdt": np.ascontiguousarray(b_dt[own][:, None]),
        "alogp": _alog_pairs(A_log[own]),
        "dskip": np.ascontiguousarray(Dskip[own][:, None]),
        "woutT": np.ascontiguousarray(W_out[:, own].T).astype(bf16),
        "selE": _selE(),
        "selR": _selR(),
    }


def _alog_pairs(alog_own):
    out = np.empty((DH, DS), np.float32)
    q = np.arange(DH)
    for p in range(DS):
        out[:, p] = alog_own[2 * p + (q % 2), q // 2]
    return out


_selE_cache = {}


def _selR():
    if "r" not in _selE_cache:
        sel = np.zeros((DH, DS * DH), np.float32)
        q = np.arange(DH)
        for p in range(DS):
            sel[2 * p + (q % 2), DH * p + q] = 1.0
        _selE_cache["r"] = sel.astype(ml_dtypes.bfloat16)
    return _selE_cache["r"]


def _selE():
    if "v" not in _selE_cache:
        sel = np.zeros((DH, DS * DH), np.float32)
        q = np.arange(DH)
        for p in range(DS):
            sel[q, DH * p + 2 * p + (q % 2)] = 1.0
        _selE_cache["v"] = sel.astype(ml_dtypes.bfloat16)
    return _selE_cache["v"]


def kernel(**inputs) -> np.ndarray:
    if "nc" not in _cache:
        _cache["nc"] = _build()
    nc = _cache["nc"]

    in_maps = [_core_inputs(inputs, c // 2, c % 2) for c in range(8)]
    res = run_bass_kernel_spmd(nc, in_maps, core_ids=list(range(8)))

    pooled = np.zeros((B, DM), np.float32)
    for c in range(8):
        pooled[c // 2] += res.results[c]["pooled"][:, 0] / L

    f32 = np.float32
    W1 = np.asarray(inputs["W1"], f32)
    b1 = np.asarray(inputs["b1"], f32)
    gamma = np.asarray(inputs["gamma"], f32)
    beta = np.asarray(inputs["beta"], f32)
    W2 = np.asarray(inputs["W2"], f32)
    b2 = np.asarray(inputs["b2"], f32)
    h1 = pooled @ W1.T + b1
    mu = h1.mean(axis=0)
    var = h1.var(axis=0)
    h1 = (h1 - mu) / np.sqrt(var + EPS) * gamma + beta
    h1 = np.maximum(h1, 0.0)
    return (h1 @ W2.T + b2).astype(np.float32)
